# revision 1
# baseline (speedup 1.0000x reference)
"""GATv2 attention head (single head) on 8 Trainium2 NeuronCores.

Math: with h = x @ W1^T + b, z = leaky_relu(h), s2 = z@a2, the GATv2 segment
softmax over src makes the s1[src] term cancel, so with p = exp(s2):

    out[i] = sum_{j in N(i) u {i}} p[j] * h[j] / sum_j p[j]

Sharding (dst-parallel): edges are bucketed by dst range (12500 nodes per
core), so each core's gather table g = [p*h, p] covers only its LOCAL dst
nodes (12800 rows -> int16 gather indices, no cross-core table exchange).
Per-edge rows are gathered with the SWDGE dma_gather ucode and segment-summed
by SRC on the PE: each gather call fills [128 slots, 15 cols, 34] and a
matmul with a host-built 0/1 lhsT profile [128, 32] adds each node's slots
into its PSUM cell.  Node cells live on a GLOBAL degree-sorted grid (tile =
32 rows x 15 cols) shared by all cores, so the per-core partial outputs align
row-for-row and one ReduceScatter (sum over the 8 cores) yields each core's
shard of the output.  Normalization (num/denom) and grid->node unpermutation
happen on the host.
"""
import sys
sys.path.insert(0, '/opt/trn_rl_repo')

import numpy as np
import jax
from jax.sharding import Mesh, PartitionSpec
from jax.experimental.shard_map import shard_map

import concourse.bass as bass
import concourse.bacc as bacc
import concourse.mybir as mybir
import concourse.tile as tile
from concourse.bass import exact_div
from concourse.masks import make_identity
from concourse import bass2jax
from concourse.bass2jax import _bass_exec_p, install_neuronx_cc_hook

F32 = mybir.dt.float32
BF16 = mybir.dt.bfloat16
I16 = mybir.dt.int16

NCORES = 8
N = 100000
DIN = 128
DOUT = 32
SLOPE = 0.2
NPC = N // NCORES            # nodes per core (12500)
NODE_PAD = 12800             # padded local table rows
GF = DOUT + 2                # table row: [p*h (32), p, 0]
GSTRIDE = 128                # bf16 elements per table row (256B)
ZROW = NODE_PAD - 1          # always-zero table row (pad-slot target)
MMCHUNK = 512
NCHUNKS = NODE_PAD // MMCHUNK
C = 15                       # grid columns per tile
K = 32                       # grid rows per tile (matmul out partitions)
NI = C * 128                 # gather idxs per tile
IPP = NI // 16               # idx stream int16 per partition per tile (120)
GROUP = 3                    # tiles per PSUM bank (offsets 0/32/64)
SUPER = 2                    # groups per idx-load/evac-DMA batch


def dma_gather_raw(nc, out_ap, in_ap, idxs_ap, num_idxs, elem_size,
                   elem_step, single_packet=False):
    """bass.dma_gather minus the elem%256 assert (only the row *stride*
    must be a 256B multiple for the ucode)."""
    assert idxs_ap.dtype == I16
    assert in_ap.dtype == out_ap.dtype
    assert in_ap.ap[0][0] == elem_step
    stride_bytes_256 = exact_div(elem_step * mybir.dt.size(in_ap.dtype), 256)
    eng = nc.gpsimd
    _in_ap = eng.lower_ap_dma(in_ap, for_custom_bir_dma=True)
    _idxs_ap = eng.lower_ap(idxs_ap)
    _out_ap = eng.lower_ap(out_ap)
    return eng.add_instruction(
        mybir.InstDMAGatherAnt(
            name=nc.get_next_instruction_name(),
            ins=[*_in_ap, _idxs_ap,
                 eng.lower_val_access(eng.to_reg(num_idxs))],
            outs=[_out_ap],
            transpose=False, num_idxs=num_idxs, elem_size=elem_size,
            stride_bytes_256=stride_bytes_256, gen_mode=0,
            single_packet=single_packet, queue_num=0, sbuf_tokens_per_rank=0,
            sbuf_free_dim_per_rank=0, sbuf_free_dim_pad_per_rank=0,
            sbuf_byte_offset=0))


def _wrap(cs):
    """Per-call idx stream layout: cs [cols, 128] -> [128, cols*8]."""
    pc = cs.shape[0]
    arr = cs.reshape(-1)                  # [pc*128] col-major over slots
    arr = arr.reshape(pc * 8, 16).T       # idx i at [i%16, i//16]
    return np.tile(arr, (8, 1))           # [128, pc*8]


def _host_shard(x, edge_index):
    """Dst-bucket edges; build global degree-sorted grid + per-core
    lhsT profiles and gather idx streams."""
    src = np.asarray(edge_index[0]).astype(np.int64)
    dst = np.asarray(edge_index[1]).astype(np.int64)
    x = np.asarray(x)

    # per (src, core) counts including self loops
    counts = np.zeros((N, NCORES), np.int32)
    np.add.at(counts, (src, dst // NPC), 1)
    counts[np.arange(N), np.arange(N) // NPC] += 1
    deg = counts.sum(1)

    order = np.argsort(-deg, kind='stable')      # sigma: global node order
    nrows = -(-N // C)
    padn = nrows * C - N
    sc = counts[order]
    scp = np.vstack([sc, np.zeros((padn, NCORES), np.int32)])
    rowneed = scp.reshape(nrows, C, NCORES).max(axis=1)   # [nrows, 8]

    # sequential greedy tiles: rows while all-core sum <= 128 and rows <= K
    tile_of_row = np.zeros(nrows, np.int64)
    row_in_tile = np.zeros(nrows, np.int64)
    cur = np.zeros(NCORES, np.int64)
    t = 0
    r_in = 0
    for r in range(nrows):
        if r_in >= K or np.any(cur + rowneed[r] > 128):
            t += 1
            cur[:] = 0
            r_in = 0
        tile_of_row[r] = t
        row_in_tile[r] = r_in
        cur += rowneed[r]
        r_in += 1
    ntiles = t + 1

    # cell map: grid row-major cell (t, i, c) -> node (or -1)
    cell_map = np.full(ntiles * K * C, -1, np.int64)
    gr = np.arange(nrows)
    nodes_p = np.concatenate([order, np.full(padn, -1, np.int64)])
    for c in range(C):
        cells = tile_of_row * (K * C) + row_in_tile * C + c
        cell_map[cells] = nodes_p[gr * C + c]

    # per-core edge lists grouped by src
    per_core_inputs = []
    lhs_all = []
    for k in range(NCORES):
        sel = np.flatnonzero(dst // NPC == k)
        es = src[sel]
        ed = dst[sel] - k * NPC
        # self loops for this core's node range
        own = np.arange(k * NPC, (k + 1) * NPC, dtype=np.int64)
        es = np.concatenate([es, own])
        ed = np.concatenate([ed, own - k * NPC])
        o2 = np.argsort(es, kind='stable')
        es, ed = es[o2], ed[o2]
        starts = np.searchsorted(es, np.arange(N + 1))

        cnt_k = counts[:, k]
        slots = np.full((ntiles, C, 128), ZROW, np.int16)
        lhsT = np.zeros((ntiles, 128, K), np.int8)
        # partition cursor per tile: allocate D = max over cols per row
        # (vectorized-ish per tile)
        for tt in range(ntiles):
            base = tt * K * C
            p0 = 0
            for i in range(K):
                cells = cell_map[base + i * C:base + i * C + C]
                valid = cells >= 0
                if not valid.any():
                    continue
                D = int(cnt_k[cells[valid]].max()) if valid.any() else 0
                if D == 0:
                    continue
                lhsT[tt, p0:p0 + D, i] = 1
                for c in np.flatnonzero(valid):
                    n = cells[c]
                    s0, s1 = starts[n], starts[n + 1]
                    m = s1 - s0
                    if m:
                        slots[tt, c, p0:p0 + m] = ed[s0:s1].astype(np.int16)
                p0 += D
            assert p0 <= 128, (tt, p0)

        # idx stream: per tile wrap -> [128, ntiles*IPP]
        stream = np.concatenate(
            [_wrap(slots[tt]) for tt in range(ntiles)], axis=1)
        # lhsT layout [128, ntiles*K] bf16
        import ml_dtypes
        lt = lhsT.transpose(1, 0, 2).reshape(128, ntiles * K)
        lt = lt.astype(ml_dtypes.bfloat16)

        xT = np.zeros((DIN, NODE_PAD), np.float32)
        xT[:, :NPC] = x[k * NPC:(k + 1) * NPC].T
        per_core_inputs.append({"xT": xT, "slots": stream, "lhs": lt})

    return ntiles, cell_map, per_core_inputs


def _build_program(ntiles):
    ngroups = -(-ntiles // GROUP)
    nsuper = -(-ngroups // SUPER)
    PADROWS = ngroups * GROUP * K * C     # partial rows (incl pad tiles)
    OUTROWS = PADROWS // NCORES
    nc = bacc.Bacc("TRN2", target_bir_lowering=False, debug=False,
                   num_devices=NCORES)
    xT_d = nc.dram_tensor("xT", [DIN, NODE_PAD], F32, kind="ExternalInput")
    w_d = nc.dram_tensor("w1t", [DIN, DOUT], F32, kind="ExternalInput")
    b_d = nc.dram_tensor("bias", [DOUT, 1], F32, kind="ExternalInput")
    a2_d = nc.dram_tensor("a2", [DOUT, GF], F32, kind="ExternalInput")
    slots_d = nc.dram_tensor("slots", [128, ntiles * IPP], I16,
                             kind="ExternalInput")
    lhs_d = nc.dram_tensor("lhs", [128, ntiles * K], BF16,
                           kind="ExternalInput")
    out_d = nc.dram_tensor("out", [OUTROWS, GF], BF16, kind="ExternalOutput")

    with tile.TileContext(nc) as tc:
        with tc.tile_pool(name="const", bufs=1) as constp, \
             tc.tile_pool(name="xp", bufs=3) as xp, \
             tc.tile_pool(name="work", bufs=3) as work, \
             tc.tile_pool(name="big", bufs=1) as big, \
             tc.tile_pool(name="gbuf", bufs=3) as gbuf, \
             tc.tile_pool(name="ibuf", bufs=3) as ibuf, \
             tc.tile_pool(name="ev", bufs=3) as evp, \
             tc.tile_pool(name="ps", bufs=2, space="PSUM") as ps, \
             tc.tile_pool(name="pst", bufs=2, space="PSUM") as pst, \
             tc.tile_pool(name="pse", bufs=2, space="PSUM") as pse, \
             tc.tile_pool(name="dram", bufs=1, space="DRAM") as dram:

            F32R = mybir.dt.float32r
            w_sb = constp.tile([DIN, DOUT], F32)
            nc.sync.dma_start(out=w_sb[:], in_=w_d[:, :])
            b_sb = constp.tile([DOUT, 1], F32)
            nc.sync.dma_start(out=b_sb[:], in_=b_d[:, :])
            a2_sb = constp.tile([DOUT, GF], F32)
            nc.sync.dma_start(out=a2_sb[:], in_=a2_d[:, :])
            lhs_sb = constp.tile([128, ntiles * K], BF16)
            nc.sync.dma_start(out=lhs_sb[:], in_=lhs_d[:, :])
            ident = constp.tile([128, 128], F32)
            make_identity(nc, ident[:])
            ones_sb = constp.tile([1, MMCHUNK], F32)
            nc.gpsimd.memset(ones_sb[:], 1.0)
            # bias row extended with a trailing 1 (becomes the p row after
            # the elementwise multiply by p)
            bb_sb = constp.tile([1, DOUT + 1], F32)
            nc.gpsimd.memset(bb_sb[0:1, DOUT:DOUT + 1], 1.0)
            nc.sync.dma_start(out=bb_sb[0:1, 0:DOUT],
                              in_=b_d[:, :].rearrange("f o -> o f"))

            # ---- node phase: gT[f, n] = [p*h; p; 0] (feature-major) ----
            gT = big.tile([GF, NODE_PAD], F32, tag="gT")
            nc.gpsimd.memset(gT[DOUT:GF, :], 0.0)
            for t in range(NCHUNKS):
                cs = slice(t * MMCHUNK, (t + 1) * MMCHUNK)
                xt = xp.tile([DIN, MMCHUNK], F32)
                nc.sync.dma_start(out=xt[:], in_=xT_d[:, cs])
                # hps = W x + b via bias accumulate on PE
                hps = ps.tile([DOUT, MMCHUNK], F32, space="PSUM")
                nc.tensor.matmul(hps[:], lhsT=w_sb[:],
                                 rhs=xt[:], start=True, stop=False)
                nc.tensor.matmul(hps[:], lhsT=bb_sb[:, 0:DOUT],
                                 rhs=ones_sb[:], start=False, stop=True)
                h_sb = work.tile([DOUT, MMCHUNK], F32, tag="h")
                nc.scalar.activation(out=h_sb[:], in_=hps[:],
                                     func=mybir.ActivationFunctionType.Copy)
                z_sb = work.tile([DOUT, MMCHUNK], F32, tag="z")
                nc.vector.scalar_tensor_tensor(
                    out=z_sb[:], in0=h_sb[0:DOUT, :], scalar=SLOPE,
                    in1=h_sb[0:DOUT, :], op0=mybir.AluOpType.mult,
                    op1=mybir.AluOpType.max)
                sps = ps.tile([GF, MMCHUNK], F32, space="PSUM", tag="s2")
                nc.tensor.matmul(sps[:], lhsT=a2_sb[:], rhs=z_sb[:],
                                 start=True, stop=True)
                p_sb = work.tile([DOUT + 1, MMCHUNK], F32, tag="p")
                nc.scalar.activation(out=p_sb[:], in_=sps[0:DOUT + 1, :],
                                     func=mybir.ActivationFunctionType.Exp)
                nc.vector.tensor_tensor(
                    out=gT[0:DOUT, cs], in0=h_sb[:], in1=p_sb[0:DOUT, :],
                    op=mybir.AluOpType.mult)
                nc.gpsimd.tensor_copy(out=gT[DOUT:DOUT + 1, cs],
                                      in_=p_sb[DOUT:DOUT + 1, :])
            # zero row (pad-slot target)
            nc.vector.memset(gT[:, ZROW:ZROW + 1], 0.0)

            # ---- transpose gT -> node-major bf16, write strided table ----
            ntile128 = NODE_PAD // 128
            g_sb = big.tile([128, ntile128 * GF], F32)
            for t in range(ntile128):
                tp = pst.tile([128, GF], F32, space="PSUM")
                nc.tensor.transpose(
                    out=tp[:], in_=gT[:, t * 128:(t + 1) * 128],
                    identity=ident[:GF, :GF])
                if t % 2 == 0:
                    nc.vector.tensor_copy(
                        out=g_sb[:, t * GF:(t + 1) * GF], in_=tp[:])
                else:
                    nc.scalar.activation(
                        out=g_sb[:, t * GF:(t + 1) * GF], in_=tp[:],
                        func=mybir.ActivationFunctionType.Copy)
            tbl = dram.tile([NODE_PAD, GSTRIDE], BF16)
            nc.gpsimd.dma_start(
                out=tbl[:, 0:GF].rearrange("(t p) f -> p t f", p=128),
                in_=g_sb[:].rearrange("p (t f) -> p t f", f=GF))

            # ---- edge phase: gather + matmul-reduce ----
            part_d = dram.tile([PADROWS, GF], BF16)
            for s in range(nsuper):
                g_lo = s * SUPER
                g_hi = min((s + 1) * SUPER, ngroups)
                ngr = g_hi - g_lo
                idx = ibuf.tile([128, ngr * GROUP * IPP], I16, tag="idx")
                t_lo = g_lo * GROUP
                n_idx_tiles = min(ngr * GROUP, ntiles - t_lo)
                nc.sync.dma_start(
                    out=idx[:, 0:n_idx_tiles * IPP],
                    in_=slots_d[:, t_lo * IPP:(t_lo + n_idx_tiles) * IPP])
                ev = evp.tile([96, ngr * 510], BF16, tag="ev")
                for gi in range(ngr):
                    g = g_lo + gi
                    out_ps = pse.tile([128, 512], F32, space="PSUM",
                                      tag="eps")
                    for j in range(GROUP):
                        t = g * GROUP + j
                        if t < ntiles:
                            B = gbuf.tile([128, C * GF], BF16, tag="B")
                            dma_gather_raw(
                                nc, B[:].rearrange("p (c f) -> p c f", f=GF),
                                tbl[:, 0:GF],
                                idx[:, (gi * GROUP + j) * IPP:
                                       (gi * GROUP + j + 1) * IPP],
                                NI, GF, GSTRIDE)
                            nc.tensor.matmul(
                                out=out_ps[j * K:(j + 1) * K, 0:C * GF],
                                lhsT=lhs_sb[:, t * K:(t + 1) * K],
                                rhs=B[:], start=True, stop=True)
                        else:
                            # pad tile: zero PSUM rows via memset
                            nc.vector.memset(
                                out_ps[j * K:(j + 1) * K, 0:C * GF], 0.0)
                    nc.scalar.activation(
                        out=ev[:, gi * 510:(gi + 1) * 510],
                        in_=out_ps[0:96, 0:C * GF],
                        func=mybir.ActivationFunctionType.Copy)
                nc.scalar.dma_start(
                    out=part_d[g_lo * GROUP * K * C:
                               (g_lo * GROUP + ngr * GROUP) * K * C, :]
                        .rearrange("(g j i c) f -> (j i) g c f",
                                   g=ngr, j=GROUP, c=C),
                    in_=ev[:].rearrange("p (g c f) -> p g c f",
                                        g=ngr, f=GF))

            # ---- ReduceScatter partials -> my output shard ----
            rs_out = dram.tile([OUTROWS, GF], BF16)
            nc.gpsimd.collective_compute(
                "ReduceScatter", mybir.AluOpType.add,
                ins=[part_d[:].opt()], outs=[rs_out[:].opt()],
                replica_groups=[list(range(NCORES))])
            nc.sync.dma_start(out=out_d[:, :], in_=rs_out[:, :])

    nc.compile()
    return nc


class _Runner:
    """shard_map-jitted executor (mirrors bass2jax.run_bass_via_pjrt)."""

    def __init__(self, nc, n_cores):
        install_neuronx_cc_hook()
        self.n_cores = n_cores
        partition_name = (nc.partition_id_tensor.name
                          if nc.partition_id_tensor else None)
        in_names, out_names, out_avals, zero_outs = [], [], [], []
        for alloc in nc.m.functions[0].allocations:
            if not isinstance(alloc, mybir.MemoryLocationSet):
                continue
            name = alloc.memorylocations[0].name
            if alloc.kind == "ExternalInput":
                if name != partition_name:
                    in_names.append(name)
            elif alloc.kind == "ExternalOutput":
                out_names.append(name)
                shape = tuple(alloc.tensor_shape)
                dtype = mybir.dt.np(alloc.dtype)
                out_avals.append(jax.core.ShapedArray(shape, dtype))
                zero_outs.append(np.zeros(shape, dtype))
        self.in_names = in_names
        self.out_names = out_names
        self.out_avals = out_avals
        self.zero_outs = zero_outs
        n_params = len(in_names)
        self.n_params = n_params
        all_in = in_names + out_names
        if partition_name is not None:
            all_in.append(partition_name)
        donate = tuple(range(n_params, n_params + len(out_avals)))

        def _body(*args):
            operands = list(args)
            if partition_name is not None:
                operands.append(bass2jax.partition_id_tensor())
            outs = _bass_exec_p.bind(
                *operands, out_avals=tuple(out_avals),
                in_names=tuple(all_in), out_names=tuple(out_names),
                lowering_input_output_aliases=(),
                sim_require_finite=True, sim_require_nnan=True, nc=nc)
            return tuple(outs)

        devices = jax.devices()[:n_cores]
        mesh = Mesh(np.asarray(devices), ("core",))
        self._fn = jax.jit(
            shard_map(_body, mesh=mesh,
                      in_specs=(PartitionSpec("core"),) * (n_params +
                                                           len(out_avals)),
                      out_specs=(PartitionSpec("core"),) * len(out_names),
                      check_rep=False),
            donate_argnums=donate, keep_unused=True)

    def run(self, in_maps):
        per_core = [[np.asarray(m[n]) for n in self.in_names]
                    for m in in_maps]
        concat_in = [
            np.concatenate([per_core[c][i] for c in range(self.n_cores)],
                           axis=0)
            for i in range(self.n_params)
        ]
        concat_zeros = [
            np.zeros((self.n_cores * z.shape[0], *z.shape[1:]), z.dtype)
            for z in self.zero_outs
        ]
        out_arrs = self._fn(*concat_in, *concat_zeros)
        jax.block_until_ready(out_arrs)
        return [
            {name: np.asarray(out_arrs[i]).reshape(
                self.n_cores, *self.out_avals[i].shape)[c]
             for i, name in enumerate(self.out_names)}
            for c in range(self.n_cores)
        ]


_CACHE = {}


def _consts(W1_w, W1_b, a2_w):
    return {
        "w1t": np.ascontiguousarray(np.asarray(W1_w).T).astype(np.float32),
        "bias": np.asarray(W1_b).reshape(DOUT, 1).astype(np.float32),
        "a2": np.repeat(np.asarray(a2_w).reshape(DOUT, 1), GF,
                        axis=1).astype(np.float32),
    }


def _get_runner(ntiles):
    if ntiles not in _CACHE:
        nc = _build_program(ntiles)
        _CACHE[ntiles] = (nc, _Runner(nc, NCORES))
    return _CACHE[ntiles]


def kernel(x, edge_index, W1_w, W1_b, a1_w=None, a2_w=None):
    ntiles, cell_map, per_core = _host_shard(x, edge_index)
    nc, runner = _get_runner(ntiles)
    consts = _consts(W1_w, W1_b, a2_w)
    in_maps = [{**per_core[c], **consts} for c in range(NCORES)]
    results = runner.run(in_maps)
    full = np.concatenate([results[c]["out"] for c in range(NCORES)], axis=0)
    ncells = ntiles * K * C
    cm = np.concatenate([cell_map,
                         np.full(full.shape[0] - ncells, -1, np.int64)])
    valid = cm >= 0
    out = np.empty((N, DOUT), np.float32)
    rows = full[valid].astype(np.float32)
    out[cm[valid]] = rows[:, 0:DOUT] / rows[:, DOUT:DOUT + 1]
    return out



# revision 24
# speedup vs baseline: 1.2756x; 1.2756x over previous
"""GATv2 attention head (single head) on 8 Trainium2 NeuronCores.

Math: with h = x @ W1^T + b, z = leaky_relu(h), s2 = z@a2, the GATv2 segment
softmax over src makes the s1[src] term cancel, so with p = exp(s2):

    out[i] = sum_{j in N(i) u {i}} p[j] * h[j] / sum_j p[j]

Sharding (dst-parallel): edges are bucketed by dst range (12500 nodes per
core), so each core's gather table g = [p*h, p] covers only its LOCAL dst
nodes (12800 rows -> int16 gather indices, no cross-core table exchange).
Per-edge rows are gathered with the SWDGE dma_gather ucode and segment-summed
by SRC on the PE: each gather call fills [128 slots, 15 cols, 33] and a
matmul with a host-built 0/1 lhsT profile [128, 32] adds each node's slots
into its PSUM cell.  Node cells live on a GLOBAL degree-sorted grid (tile =
32 rows x 15 cols) shared by all cores, so the per-core partial outputs align
row-for-row and one ReduceScatter (sum over the 8 cores) yields each core's
shard of the output.  Normalization (num/denom) and grid->node unpermutation
happen on the host.

Perf notes (CoreSim v1 cost model):
 - gathers/evacs are priced per output-element-per-partition on the issuing
   engine; they are split across Pool/Act/DVE via a static greedy balancer.
 - node phase is all-bf16 (fp32 matmuls cost 4x on PE).
 - the ReduceScatter writes straight into the output tensor.
"""
import sys
sys.path.insert(0, '/opt/trn_rl_repo')

import numpy as np
import jax
from jax.sharding import Mesh, PartitionSpec
from jax.experimental.shard_map import shard_map

import concourse.bass as bass
import concourse.bacc as bacc
import concourse.mybir as mybir
import concourse.tile as tile
from concourse.bass import exact_div
from concourse.masks import make_identity
from concourse import bass2jax
from concourse.bass2jax import _bass_exec_p, install_neuronx_cc_hook

F32 = mybir.dt.float32
BF16 = mybir.dt.bfloat16
I16 = mybir.dt.int16

NCORES = 8
N = 100000
DIN = 128
DOUT = 32
SLOPE = 0.2
NPC = N // NCORES            # nodes per core (12500)
NODE_PAD = 12800             # padded local table rows
GF = DOUT + 1                # table row: [p*h (32), p]
GSTRIDE = 128                # bf16 elements per table row (256B)
ZROW = NODE_PAD - 1          # always-zero table row (pad-slot target)
MMCHUNK = 512
NCHUNKS = NODE_PAD // MMCHUNK
C = 15                       # grid columns per tile
K = 32                       # grid rows per tile (matmul out partitions)
NI = C * 128                 # gather idxs per tile
IPP = NI // 16               # idx stream int16 per partition per tile (120)
GROUP = 3                    # tiles per PSUM bank (offsets 0/32/64)
SUPER = 2                    # groups per idx-load/evac-DMA batch

POOL, ACT, DVE, SP = 0, 1, 2, 3


class _Balance:
    """Static greedy load balancer over engine queues (build-time)."""

    def __init__(self, nc):
        self.eng = [nc.gpsimd, nc.scalar, nc.vector, nc.sync]
        self.busy = [0.0, 0.0, 0.0, 0.0]

    def pick(self, costs):
        """costs: dict {engine_id: est_ns}; returns (bass_engine, id)."""
        best = min(costs, key=lambda e: self.busy[e] + costs[e])
        self.busy[best] += costs[best]
        return self.eng[best], best


def dma_gather_raw(nc, out_ap, in_ap, idxs_ap, num_idxs, elem_size,
                   elem_step, single_packet=False, eng=None):
    """bass.dma_gather minus the elem%256 assert (only the row *stride*
    must be a 256B multiple for the ucode)."""
    assert idxs_ap.dtype == I16
    assert in_ap.dtype == out_ap.dtype
    assert in_ap.ap[0][0] == elem_step
    stride_bytes_256 = exact_div(elem_step * mybir.dt.size(in_ap.dtype), 256)
    if eng is None:
        eng = nc.gpsimd
    _in_ap = eng.lower_ap_dma(in_ap, for_custom_bir_dma=True)
    _idxs_ap = eng.lower_ap(idxs_ap)
    _out_ap = eng.lower_ap(out_ap)
    return eng.add_instruction(
        mybir.InstDMAGatherAnt(
            name=nc.get_next_instruction_name(),
            ins=[*_in_ap, _idxs_ap,
                 eng.lower_val_access(eng.to_reg(num_idxs))],
            outs=[_out_ap],
            transpose=False, num_idxs=num_idxs, elem_size=elem_size,
            stride_bytes_256=stride_bytes_256, gen_mode=0,
            single_packet=single_packet, queue_num=0, sbuf_tokens_per_rank=0,
            sbuf_free_dim_per_rank=0, sbuf_free_dim_pad_per_rank=0,
            sbuf_byte_offset=0))


def _wrap(cs):
    """Per-call idx stream layout: cs [cols, 128] -> [128, cols*8]."""
    pc = cs.shape[0]
    arr = cs.reshape(-1)                  # [pc*128] col-major over slots
    arr = arr.reshape(pc * 8, 16).T       # idx i at [i%16, i//16]
    return np.tile(arr, (8, 1))           # [128, pc*8]


def _host_shard(x, edge_index):
    """Dst-bucket edges; build global degree-sorted grid + per-core
    lhsT profiles and gather idx streams."""
    import ml_dtypes
    src = np.asarray(edge_index[0]).astype(np.int64)
    dst = np.asarray(edge_index[1]).astype(np.int64)
    x = np.asarray(x)

    # per (src, core) counts including self loops
    counts = np.zeros((N, NCORES), np.int32)
    np.add.at(counts, (src, dst // NPC), 1)
    counts[np.arange(N), np.arange(N) // NPC] += 1
    deg = counts.sum(1)

    # --- greedy row clustering: group nodes with similar per-core count
    # vectors into rows of C, minimizing sum-of-core-maxes padding ---
    R = 256
    cmax = counts.max(1)
    proc_order = np.argsort(-cmax, kind='stable')
    rows_cur = np.zeros((R, NCORES), np.int32)
    rows_fill = np.zeros(R, np.int32)
    row_members = [[] for _ in range(R)]
    finished = []
    for n in proc_order:
        cv = counts[n]
        delta = (np.maximum(rows_cur, cv) - rows_cur).sum(1)
        cand = int(np.argmin(delta))
        rows_cur[cand] = np.maximum(rows_cur[cand], cv)
        row_members[cand].append(n)
        rows_fill[cand] += 1
        if rows_fill[cand] == C:
            finished.append(row_members[cand])
            row_members[cand] = []
            rows_fill[cand] = 0
            rows_cur[cand] = 0
    for m in row_members:
        if m:
            finished.append(m)
    rowneed = np.array([
        counts[m].max(0) if len(m) == C else
        np.vstack([counts[m],
                   np.zeros((C - len(m), NCORES), np.int32)]).max(0)
        for m in finished])
    row_order = np.argsort(-rowneed.sum(1), kind='stable')
    rowneed = rowneed[row_order]
    finished = [finished[r] for r in row_order]
    nrows = len(finished)

    # sequential greedy tiles: rows while all-core sum <= 128 and rows <= K
    tile_of_row = np.zeros(nrows, np.int64)
    row_in_tile = np.zeros(nrows, np.int64)
    cur = np.zeros(NCORES, np.int64)
    t = 0
    r_in = 0
    for r in range(nrows):
        if r_in >= K or np.any(cur + rowneed[r] > 128):
            t += 1
            cur[:] = 0
            r_in = 0
        tile_of_row[r] = t
        row_in_tile[r] = r_in
        cur += rowneed[r]
        r_in += 1
    ntiles = t + 1

    # cell map: grid row-major cell (t, i, c) -> node (or -1)
    cell_map = np.full(ntiles * K * C, -1, np.int64)
    for r in range(nrows):
        base = tile_of_row[r] * (K * C) + row_in_tile[r] * C
        m = finished[r]
        cell_map[base:base + len(m)] = m

    # per-core edge lists grouped by src
    per_core_inputs = []
    for k in range(NCORES):
        sel = np.flatnonzero(dst // NPC == k)
        es = src[sel]
        ed = dst[sel] - k * NPC
        # self loops for this core's node range
        own = np.arange(k * NPC, (k + 1) * NPC, dtype=np.int64)
        es = np.concatenate([es, own])
        ed = np.concatenate([ed, own - k * NPC])
        o2 = np.argsort(es, kind='stable')
        es, ed = es[o2], ed[o2]
        starts = np.searchsorted(es, np.arange(N + 1))

        cnt_k = counts[:, k]
        slots = np.full((ntiles, C, 128), ZROW, np.int16)
        lhsT = np.zeros((ntiles, 128, K), np.int8)
        # partition cursor per tile: allocate D = max over cols per row
        for tt in range(ntiles):
            base = tt * K * C
            p0 = 0
            for i in range(K):
                cells = cell_map[base + i * C:base + i * C + C]
                valid = cells >= 0
                if not valid.any():
                    continue
                D = int(cnt_k[cells[valid]].max()) if valid.any() else 0
                if D == 0:
                    continue
                lhsT[tt, p0:p0 + D, i] = 1
                for c in np.flatnonzero(valid):
                    n = cells[c]
                    s0, s1 = starts[n], starts[n + 1]
                    m = s1 - s0
                    if m:
                        slots[tt, c, p0:p0 + m] = ed[s0:s1].astype(np.int16)
                p0 += D
            assert p0 <= 128, (tt, p0)

        # idx stream: per tile wrap -> [128, ntiles*IPP]
        stream = np.concatenate(
            [_wrap(slots[tt]) for tt in range(ntiles)], axis=1)
        # lhsT layout [128, ntiles*K] bf16
        lt = lhsT.transpose(1, 0, 2).reshape(128, ntiles * K)
        lt = lt.astype(ml_dtypes.bfloat16)

        xT = np.zeros((DIN, NODE_PAD), ml_dtypes.bfloat16)
        xT[:, :NPC] = x[k * NPC:(k + 1) * NPC].T.astype(ml_dtypes.bfloat16)
        per_core_inputs.append({"xT": xT, "slots": stream, "lhs": lt})

    return ntiles, cell_map, per_core_inputs


def _build_program(ntiles):
    ngroups = -(-ntiles // GROUP)
    nsuper = -(-ngroups // SUPER)
    PADROWS = ngroups * GROUP * K * C     # partial rows (incl pad tiles)
    OUTROWS = PADROWS // NCORES
    nc = bacc.Bacc("TRN2", target_bir_lowering=False, debug=False,
                   num_devices=NCORES)
    xT_d = nc.dram_tensor("xT", [DIN, NODE_PAD], BF16, kind="ExternalInput")
    w_d = nc.dram_tensor("w33", [DIN, GF], BF16, kind="ExternalInput")
    b_d = nc.dram_tensor("b33", [1, GF], BF16, kind="ExternalInput")
    a2_d = nc.dram_tensor("a233", [DOUT, GF], BF16, kind="ExternalInput")
    wa2_d = nc.dram_tensor("wa233", [DIN, GF], BF16, kind="ExternalInput")
    c_d = nc.dram_tensor("c33", [1, GF], BF16, kind="ExternalInput")
    slots_d = nc.dram_tensor("slots", [128, ntiles * IPP], I16,
                             kind="ExternalInput")
    lhs_d = nc.dram_tensor("lhs", [128, ntiles * K], BF16,
                           kind="ExternalInput")
    out_d = nc.dram_tensor("out", [OUTROWS, GF], BF16, kind="ExternalOutput")

    with tile.TileContext(nc) as tc:
        with tc.tile_pool(name="const", bufs=1) as constp, \
             tc.tile_pool(name="xp", bufs=3) as xp, \
             tc.tile_pool(name="work", bufs=3) as work, \
             tc.tile_pool(name="big", bufs=1) as big, \
             tc.tile_pool(name="gbuf", bufs=6) as gbuf, \
             tc.tile_pool(name="ibuf", bufs=3) as ibuf, \
             tc.tile_pool(name="ev", bufs=3) as evp, \
             tc.tile_pool(name="ps", bufs=2, space="PSUM") as ps, \
             tc.tile_pool(name="pst", bufs=2, space="PSUM") as pst, \
             tc.tile_pool(name="pse", bufs=2, space="PSUM") as pse, \
             tc.tile_pool(name="pse2", bufs=2, space="PSUM") as pse2, \
             tc.tile_pool(name="dram", bufs=1, space="DRAM") as dram:

            # ---- consts, one per queue so they load in parallel ----
            w_sb = constp.tile([DIN, GF], BF16)
            nc.sync.dma_start(out=w_sb[:], in_=w_d[:, :])
            b_sb = constp.tile([1, GF], BF16)
            nc.scalar.dma_start(out=b_sb[:], in_=b_d[:, :])
            a2_sb = constp.tile([DOUT, GF], BF16)
            nc.gpsimd.dma_start(out=a2_sb[:], in_=a2_d[:, :])
            wa2_sb = constp.tile([DIN, GF], BF16)
            nc.gpsimd.dma_start(out=wa2_sb[:], in_=wa2_d[:, :])
            c_sb = constp.tile([1, GF], BF16)
            nc.sync.dma_start(out=c_sb[:], in_=c_d[:, :])
            ident = constp.tile([128, 128], BF16)
            make_identity(nc, ident[:])
            ones_sb = constp.tile([1, MMCHUNK], BF16)
            nc.gpsimd.memset(ones_sb[:], 1.0)
            # edge-phase profiles: split the load across three queues
            lhs_sb = constp.tile([128, ntiles * K], BF16)
            third = (ntiles * K) // 3
            nc.sync.dma_start(out=lhs_sb[:, 0:third],
                              in_=lhs_d[:, 0:third])
            nc.scalar.dma_start(out=lhs_sb[:, third:2 * third],
                                in_=lhs_d[:, third:2 * third])
            nc.gpsimd.dma_start(out=lhs_sb[:, 2 * third:],
                                in_=lhs_d[:, 2 * third:])

            # ---- node phase: gT[f, n] = [p*h; p] (feature-major, bf16) ----
            # hps rows 0:32 = W@x + b, row 32 = 1.0 (w col 32 = 0, b col 32
            # = 1), so gT = hps * p has row 32 = p.
            gT = big.tile([GF, NODE_PAD], BF16, tag="gT")
            for t in range(NCHUNKS):
                cs = slice(t * MMCHUNK, (t + 1) * MMCHUNK)
                xt = xp.tile([DIN, MMCHUNK], BF16)
                nc.sync.dma_start(out=xt[:], in_=xT_d[:, cs])
                hps = ps.tile([GF, MMCHUNK], F32, space="PSUM")
                nc.tensor.matmul(hps[:], lhsT=w_sb[:],
                                 rhs=xt[:], start=True, stop=False)
                nc.tensor.matmul(hps[:], lhsT=b_sb[:],
                                 rhs=ones_sb[:], start=False, stop=True)
                # leaky(h) = slope*h + (1-slope)*relu(h); the slope*h term
                # is linear in x so it folds into the s2 matmul via
                # wa2 = W^T a2 (host-precomputed const):
                # s2 = slope*(wa2^T x) + (1-slope)*(a2^T relu(h)) + slope*a2.b
                r_sb = work.tile([DOUT, MMCHUNK], BF16, tag="r")
                nc.scalar.activation(out=r_sb[:], in_=hps[0:DOUT, :],
                                     func=mybir.ActivationFunctionType.Relu)
                sps = pse.tile([GF, MMCHUNK], F32, space="PSUM", tag="s2")
                nc.tensor.matmul(sps[:], lhsT=wa2_sb[:], rhs=xt[:],
                                 start=True, stop=False)
                nc.tensor.matmul(sps[:], lhsT=a2_sb[:], rhs=r_sb[:],
                                 start=False, stop=False)
                nc.tensor.matmul(sps[:], lhsT=c_sb[:], rhs=ones_sb[:],
                                 start=False, stop=True)
                p_sb = work.tile([GF, MMCHUNK], BF16, tag="p")
                nc.scalar.activation(out=p_sb[:], in_=sps[:],
                                     func=mybir.ActivationFunctionType.Exp)
                nc.vector.tensor_tensor(
                    out=gT[:, cs], in0=hps[:], in1=p_sb[:],
                    op=mybir.AluOpType.mult)
            # zero pad columns (incl ZROW, the pad-slot target)
            nc.vector.memset(gT[:, NPC:NODE_PAD], 0.0)

            # ---- transpose gT -> node-major bf16, write strided table ----
            ntile128 = NODE_PAD // 128
            g_sb = big.tile([128, ntile128 * GF], BF16)
            # pack 10 transposes into one PSUM tile, evac with one wide copy
            # pack 10 transposes into one PSUM tile (stride 34 to keep PSUM
            # writes 4B-aligned), evac with one wide strided copy
            TB = 10
            GFP = GF + 1
            for tb in range(ntile128 // TB):
                tp = pst.tile([128, TB * GFP], BF16, space="PSUM")
                for u in range(TB):
                    t = tb * TB + u
                    nc.tensor.transpose(
                        out=tp[:, u * GFP:u * GFP + GF],
                        in_=gT[:, t * 128:(t + 1) * 128],
                        identity=ident[:GF, :GF])
                base = tb * TB * GF
                src = tp[:].rearrange("p (t f) -> p t f", f=GFP)[:, :, 0:GF]
                dst = g_sb[:, base:base + TB * GF].rearrange(
                    "p (t f) -> p t f", f=GF)
                if tb % 2 == 0:
                    nc.vector.tensor_copy(out=dst, in_=src)
                else:
                    nc.scalar.activation(
                        out=dst, in_=src,
                        func=mybir.ActivationFunctionType.Copy)
            tbl = dram.tile([NODE_PAD, GSTRIDE], BF16)
            half = ntile128 // 2
            nc.sync.dma_start(
                out=tbl[0:half * 128, 0:GF]
                    .rearrange("(t p) f -> p t f", p=128),
                in_=g_sb[:, 0:half * GF]
                    .rearrange("p (t f) -> p t f", f=GF))
            nc.scalar.dma_start(
                out=tbl[half * 128:, 0:GF]
                    .rearrange("(t p) f -> p t f", p=128),
                in_=g_sb[:, half * GF:]
                    .rearrange("p (t f) -> p t f", f=GF))

            # ---- edge phase: gather + matmul-reduce ----
            bal = _Balance(nc)
            GATHER_COST = {POOL: 413}   # SWDGE ucode is Pool-only on HW
            EVAC_COST = {ACT: 597, DVE: 641}
            DMA_COST = {ACT: 770, SP: 770}   # keep Pool free for gathers
            part_d = dram.tile([PADROWS, GF], BF16)
            for s in range(nsuper):
                g_lo = s * SUPER
                g_hi = min((s + 1) * SUPER, ngroups)
                ngr = g_hi - g_lo
                idx = ibuf.tile([128, ngr * GROUP * IPP], I16, tag="idx")
                t_lo = g_lo * GROUP
                n_idx_tiles = min(ngr * GROUP, ntiles - t_lo)
                eng, _ = bal.pick(DMA_COST)
                eng.dma_start(
                    out=idx[:, 0:n_idx_tiles * IPP],
                    in_=slots_d[:, t_lo * IPP:(t_lo + n_idx_tiles) * IPP])
                ev = evp.tile([96, ngr * C * GF], BF16, tag="ev")
                for gi in range(ngr):
                    g = g_lo + gi
                    out_ps = pse2.tile([128, 512], F32, space="PSUM",
                                       tag="eps")
                    for j in range(GROUP):
                        t = g * GROUP + j
                        if t < ntiles:
                            B = gbuf.tile([128, C * GF], BF16, tag="B")
                            geng, _ = bal.pick(GATHER_COST)
                            dma_gather_raw(
                                nc, B[:].rearrange("p (c f) -> p c f", f=GF),
                                tbl[:, 0:GF],
                                idx[:, (gi * GROUP + j) * IPP:
                                       (gi * GROUP + j + 1) * IPP],
                                NI, GF, GSTRIDE, eng=geng)
                            nc.tensor.matmul(
                                out=out_ps[j * K:(j + 1) * K, 0:C * GF],
                                lhsT=lhs_sb[:, t * K:(t + 1) * K],
                                rhs=B[:], start=True, stop=True)
                        else:
                            nc.vector.memset(
                                out_ps[j * K:(j + 1) * K, 0:C * GF], 0.0)
                    eeng, eid = bal.pick(EVAC_COST)
                    dst = ev[:, gi * C * GF:(gi + 1) * C * GF]
                    if eid == ACT:
                        nc.scalar.activation(
                            out=dst, in_=out_ps[0:96, 0:C * GF],
                            func=mybir.ActivationFunctionType.Copy)
                    else:
                        eeng.tensor_copy(out=dst, in_=out_ps[0:96, 0:C * GF])
                eng, _ = bal.pick(DMA_COST)
                eng.dma_start(
                    out=part_d[g_lo * GROUP * K * C:
                               (g_lo + ngr) * GROUP * K * C, :]
                        .rearrange("(g j i c) f -> (j i) g c f",
                                   g=ngr, j=GROUP, c=C),
                    in_=ev[:].rearrange("p (g c f) -> p g c f",
                                        g=ngr, f=GF))

            # ---- ReduceScatter partials -> my output shard ----
            rs_out = dram.tile([OUTROWS, GF], BF16)
            nc.gpsimd.collective_compute(
                "ReduceScatter", mybir.AluOpType.add,
                ins=[part_d[:].opt()], outs=[rs_out[:].opt()],
                replica_groups=[list(range(NCORES))])
            # contiguous per-partition runs -> cheap DRAM->DRAM copy
            pdim = max(d for d in range(1, 129) if OUTROWS % d == 0)
            nc.sync.dma_start(
                out=out_d[:, :].rearrange("(p c) f -> p (c f)", p=pdim),
                in_=rs_out[:, :].rearrange("(p c) f -> p (c f)", p=pdim))

    nc.compile()
    return nc


class _Runner:
    """shard_map-jitted executor (mirrors bass2jax.run_bass_via_pjrt)."""

    def __init__(self, nc, n_cores):
        install_neuronx_cc_hook()
        self.n_cores = n_cores
        partition_name = (nc.partition_id_tensor.name
                          if nc.partition_id_tensor else None)
        in_names, out_names, out_avals, zero_outs = [], [], [], []
        for alloc in nc.m.functions[0].allocations:
            if not isinstance(alloc, mybir.MemoryLocationSet):
                continue
            name = alloc.memorylocations[0].name
            if alloc.kind == "ExternalInput":
                if name != partition_name:
                    in_names.append(name)
            elif alloc.kind == "ExternalOutput":
                out_names.append(name)
                shape = tuple(alloc.tensor_shape)
                dtype = mybir.dt.np(alloc.dtype)
                out_avals.append(jax.core.ShapedArray(shape, dtype))
                zero_outs.append(np.zeros(shape, dtype))
        self.in_names = in_names
        self.out_names = out_names
        self.out_avals = out_avals
        self.zero_outs = zero_outs
        n_params = len(in_names)
        self.n_params = n_params
        all_in = in_names + out_names
        if partition_name is not None:
            all_in.append(partition_name)
        donate = tuple(range(n_params, n_params + len(out_avals)))

        def _body(*args):
            operands = list(args)
            if partition_name is not None:
                operands.append(bass2jax.partition_id_tensor())
            outs = _bass_exec_p.bind(
                *operands, out_avals=tuple(out_avals),
                in_names=tuple(all_in), out_names=tuple(out_names),
                lowering_input_output_aliases=(),
                sim_require_finite=True, sim_require_nnan=True, nc=nc)
            return tuple(outs)

        devices = jax.devices()[:n_cores]
        mesh = Mesh(np.asarray(devices), ("core",))
        self._fn = jax.jit(
            shard_map(_body, mesh=mesh,
                      in_specs=(PartitionSpec("core"),) * (n_params +
                                                           len(out_avals)),
                      out_specs=(PartitionSpec("core"),) * len(out_names),
                      check_rep=False),
            donate_argnums=donate, keep_unused=True)

    def run(self, in_maps):
        per_core = [[np.asarray(m[n]) for n in self.in_names]
                    for m in in_maps]
        concat_in = [
            np.concatenate([per_core[c][i] for c in range(self.n_cores)],
                           axis=0)
            for i in range(self.n_params)
        ]
        concat_zeros = [
            np.zeros((self.n_cores * z.shape[0], *z.shape[1:]), z.dtype)
            for z in self.zero_outs
        ]
        out_arrs = self._fn(*concat_in, *concat_zeros)
        jax.block_until_ready(out_arrs)
        return [
            {name: np.asarray(out_arrs[i]).reshape(
                self.n_cores, *self.out_avals[i].shape)[c]
             for i, name in enumerate(self.out_names)}
            for c in range(self.n_cores)
        ]


_CACHE = {}


def _consts(W1_w, W1_b, a2_w):
    import ml_dtypes
    W = np.asarray(W1_w).astype(np.float32)
    b = np.asarray(W1_b).astype(np.float32)
    a2 = np.asarray(a2_w).astype(np.float32)
    w33 = np.zeros((DIN, GF), np.float32)
    w33[:, 0:DOUT] = W.T
    b33 = np.zeros((1, GF), np.float32)
    b33[0, 0:DOUT] = b
    b33[0, DOUT] = 1.0
    a233 = np.repeat(((1.0 - SLOPE) * a2).reshape(DOUT, 1), GF, axis=1)
    wa2 = SLOPE * (W.T @ a2)                       # [DIN]
    wa233 = np.repeat(wa2.reshape(DIN, 1), GF, axis=1)
    c33 = np.full((1, GF), SLOPE * float(b @ a2), np.float32)
    return {
        "w33": w33.astype(ml_dtypes.bfloat16),
        "b33": b33.astype(ml_dtypes.bfloat16),
        "a233": a233.astype(ml_dtypes.bfloat16),
        "wa233": wa233.astype(ml_dtypes.bfloat16),
        "c33": c33.astype(ml_dtypes.bfloat16),
    }


def _get_runner(ntiles):
    if ntiles not in _CACHE:
        nc = _build_program(ntiles)
        _CACHE[ntiles] = (nc, _Runner(nc, NCORES))
    return _CACHE[ntiles]


def kernel(x, edge_index, W1_w, W1_b, a1_w=None, a2_w=None):
    ntiles, cell_map, per_core = _host_shard(x, edge_index)
    nc, runner = _get_runner(ntiles)
    consts = _consts(W1_w, W1_b, a2_w)
    in_maps = [{**per_core[c], **consts} for c in range(NCORES)]
    results = runner.run(in_maps)
    full = np.concatenate([results[c]["out"] for c in range(NCORES)], axis=0)
    ncells = ntiles * K * C
    cm = np.concatenate([cell_map,
                         np.full(full.shape[0] - ncells, -1, np.int64)])
    valid = cm >= 0
    out = np.empty((N, DOUT), np.float32)
    rows = full[valid].astype(np.float32)
    out[cm[valid]] = rows[:, 0:DOUT] / rows[:, DOUT:DOUT + 1]
    return out


# revision 25
# speedup vs baseline: 1.5927x; 1.2486x over previous
"""GATv2 attention head (single head) on 8 Trainium2 NeuronCores.

Math: with h = x @ W1^T + b, z = leaky_relu(h), s2 = z@a2, the GATv2 segment
softmax over src makes the s1[src] term cancel, so with p = exp(s2):

    out[i] = sum_{j in N(i) u {i}} p[j] * h[j] / sum_j p[j]

Sharding (dst-parallel): edges are bucketed by dst range (12500 nodes per
core), so each core's gather table g = [p*h, p] covers only its LOCAL dst
nodes (12800 rows -> int16 gather indices, no cross-core table exchange).
Per-edge rows are gathered with the SWDGE dma_gather ucode and segment-summed
by SRC on the PE: each gather call fills [128 slots, 15 cols, 33] and a
matmul with a host-built 0/1 lhsT profile [128, 32] adds each node's slots
into its PSUM cell.  Node cells live on a GLOBAL degree-sorted grid (tile =
32 rows x 15 cols) shared by all cores, so the per-core partial outputs align
row-for-row and one ReduceScatter (sum over the 8 cores) yields each core's
shard of the output.  Normalization (num/denom) and grid->node unpermutation
happen on the host.

Perf notes (CoreSim v1 cost model):
 - gathers/evacs are priced per output-element-per-partition on the issuing
   engine; they are split across Pool/Act/DVE via a static greedy balancer.
 - node phase is all-bf16 (fp32 matmuls cost 4x on PE).
 - the ReduceScatter writes straight into the output tensor.
"""
import sys
sys.path.insert(0, '/opt/trn_rl_repo')

import numpy as np
import jax
from jax.sharding import Mesh, PartitionSpec
from jax.experimental.shard_map import shard_map

import concourse.bass as bass
import concourse.bacc as bacc
import concourse.mybir as mybir
import concourse.tile as tile
from concourse.bass import exact_div
from concourse.masks import make_identity
from concourse import bass2jax
from concourse.bass2jax import _bass_exec_p, install_neuronx_cc_hook

F32 = mybir.dt.float32
BF16 = mybir.dt.bfloat16
I16 = mybir.dt.int16

NCORES = 8
N = 100000
DIN = 128
DOUT = 32
SLOPE = 0.2
NPC = N // NCORES            # nodes per core (12500)
NODE_PAD = 12800             # padded local table rows
GF = DOUT + 1                # table row: [p*h (32), p]
GSTRIDE = 128                # bf16 elements per table row (256B)
ZROW = NODE_PAD - 1          # always-zero table row (pad-slot target)
MMCHUNK = 512
NCHUNKS = NODE_PAD // MMCHUNK
C = 15                       # grid columns per tile
K = 32                       # grid rows per tile (matmul out partitions)
NI = C * 128                 # gather idxs per tile
IPP = NI // 16               # idx stream int16 per partition per tile (120)
GROUP = 3                    # tiles per PSUM bank (offsets 0/32/64)
SUPER = 2                    # groups per idx-load/evac-DMA batch

POOL, ACT, DVE, SP = 0, 1, 2, 3


class _Balance:
    """Static greedy load balancer over engine queues (build-time)."""

    def __init__(self, nc):
        self.eng = [nc.gpsimd, nc.scalar, nc.vector, nc.sync]
        self.busy = [0.0, 0.0, 0.0, 0.0]

    def pick(self, costs):
        """costs: dict {engine_id: est_ns}; returns (bass_engine, id)."""
        best = min(costs, key=lambda e: self.busy[e] + costs[e])
        self.busy[best] += costs[best]
        return self.eng[best], best


def dma_gather_raw(nc, out_ap, in_ap, idxs_ap, num_idxs, elem_size,
                   elem_step, single_packet=False, eng=None):
    """bass.dma_gather minus the elem%256 assert (only the row *stride*
    must be a 256B multiple for the ucode)."""
    assert idxs_ap.dtype == I16
    assert in_ap.dtype == out_ap.dtype
    assert in_ap.ap[0][0] == elem_step
    stride_bytes_256 = exact_div(elem_step * mybir.dt.size(in_ap.dtype), 256)
    if eng is None:
        eng = nc.gpsimd
    _in_ap = eng.lower_ap_dma(in_ap, for_custom_bir_dma=True)
    _idxs_ap = eng.lower_ap(idxs_ap)
    _out_ap = eng.lower_ap(out_ap)
    return eng.add_instruction(
        mybir.InstDMAGatherAnt(
            name=nc.get_next_instruction_name(),
            ins=[*_in_ap, _idxs_ap,
                 eng.lower_val_access(eng.to_reg(num_idxs))],
            outs=[_out_ap],
            transpose=False, num_idxs=num_idxs, elem_size=elem_size,
            stride_bytes_256=stride_bytes_256, gen_mode=0,
            single_packet=single_packet, queue_num=0, sbuf_tokens_per_rank=0,
            sbuf_free_dim_per_rank=0, sbuf_free_dim_pad_per_rank=0,
            sbuf_byte_offset=0))


def _wrap(cs):
    """Per-call idx stream layout: cs [cols, 128] -> [128, cols*8]."""
    pc = cs.shape[0]
    arr = cs.reshape(-1)                  # [pc*128] col-major over slots
    arr = arr.reshape(pc * 8, 16).T       # idx i at [i%16, i//16]
    return np.tile(arr, (8, 1))           # [128, pc*8]


def _host_shard(x, edge_index):
    """Dst-bucket edges; build global degree-sorted grid + per-core
    lhsT profiles and gather idx streams."""
    import ml_dtypes
    src = np.asarray(edge_index[0]).astype(np.int64)
    dst = np.asarray(edge_index[1]).astype(np.int64)
    x = np.asarray(x)

    # per (src, core) counts including self loops
    counts = np.zeros((N, NCORES), np.int32)
    np.add.at(counts, (src, dst // NPC), 1)
    counts[np.arange(N), np.arange(N) // NPC] += 1
    deg = counts.sum(1)

    # --- greedy row clustering: group nodes with similar per-core count
    # vectors into rows of C, minimizing sum-of-core-maxes padding ---
    R = 256
    cmax = counts.max(1)
    proc_order = np.argsort(-cmax, kind='stable')
    rows_cur = np.zeros((R, NCORES), np.int32)
    rows_fill = np.zeros(R, np.int32)
    row_members = [[] for _ in range(R)]
    finished = []
    for n in proc_order:
        cv = counts[n]
        delta = (np.maximum(rows_cur, cv) - rows_cur).sum(1)
        cand = int(np.argmin(delta))
        rows_cur[cand] = np.maximum(rows_cur[cand], cv)
        row_members[cand].append(n)
        rows_fill[cand] += 1
        if rows_fill[cand] == C:
            finished.append(row_members[cand])
            row_members[cand] = []
            rows_fill[cand] = 0
            rows_cur[cand] = 0
    for m in row_members:
        if m:
            finished.append(m)
    rowneed = np.array([
        counts[m].max(0) if len(m) == C else
        np.vstack([counts[m],
                   np.zeros((C - len(m), NCORES), np.int32)]).max(0)
        for m in finished])
    row_order = np.argsort(-rowneed.sum(1), kind='stable')
    rowneed = rowneed[row_order]
    finished = [finished[r] for r in row_order]
    nrows = len(finished)

    # sequential greedy tiles: rows while all-core sum <= 128 and rows <= K
    tile_of_row = np.zeros(nrows, np.int64)
    row_in_tile = np.zeros(nrows, np.int64)
    cur = np.zeros(NCORES, np.int64)
    t = 0
    r_in = 0
    for r in range(nrows):
        if r_in >= K or np.any(cur + rowneed[r] > 128):
            t += 1
            cur[:] = 0
            r_in = 0
        tile_of_row[r] = t
        row_in_tile[r] = r_in
        cur += rowneed[r]
        r_in += 1
    ntiles = t + 1

    # cell map: grid row-major cell (t, i, c) -> node (or -1)
    cell_map = np.full(ntiles * K * C, -1, np.int64)
    for r in range(nrows):
        base = tile_of_row[r] * (K * C) + row_in_tile[r] * C
        m = finished[r]
        cell_map[base:base + len(m)] = m

    # per-core edge lists grouped by src
    per_core_inputs = []
    for k in range(NCORES):
        sel = np.flatnonzero(dst // NPC == k)
        es = src[sel]
        ed = dst[sel] - k * NPC
        # self loops for this core's node range
        own = np.arange(k * NPC, (k + 1) * NPC, dtype=np.int64)
        es = np.concatenate([es, own])
        ed = np.concatenate([ed, own - k * NPC])
        o2 = np.argsort(es, kind='stable')
        es, ed = es[o2], ed[o2]
        starts = np.searchsorted(es, np.arange(N + 1))

        cnt_k = counts[:, k]
        slots = np.full((ntiles, C, 128), ZROW, np.int16)
        lhsT = np.zeros((ntiles, 128, K), np.int8)
        # partition cursor per tile: allocate D = max over cols per row
        for tt in range(ntiles):
            base = tt * K * C
            p0 = 0
            for i in range(K):
                cells = cell_map[base + i * C:base + i * C + C]
                valid = cells >= 0
                if not valid.any():
                    continue
                D = int(cnt_k[cells[valid]].max()) if valid.any() else 0
                if D == 0:
                    continue
                lhsT[tt, p0:p0 + D, i] = 1
                for c in np.flatnonzero(valid):
                    n = cells[c]
                    s0, s1 = starts[n], starts[n + 1]
                    m = s1 - s0
                    if m:
                        slots[tt, c, p0:p0 + m] = ed[s0:s1].astype(np.int16)
                p0 += D
            assert p0 <= 128, (tt, p0)

        # idx stream: per tile wrap -> [128, ntiles*IPP]
        stream = np.concatenate(
            [_wrap(slots[tt]) for tt in range(ntiles)], axis=1)
        # lhsT layout [128, ntiles*K] bf16
        lt = lhsT.transpose(1, 0, 2).reshape(128, ntiles * K)
        lt = lt.astype(ml_dtypes.bfloat16)

        xT = np.zeros((DIN, NODE_PAD), ml_dtypes.bfloat16)
        xT[:, :NPC] = x[k * NPC:(k + 1) * NPC].T.astype(ml_dtypes.bfloat16)
        per_core_inputs.append({"xT": xT, "slots": stream, "lhs": lt})

    return ntiles, cell_map, per_core_inputs


def _build_program(ntiles):
    ngroups = -(-ntiles // GROUP)
    nsuper = -(-ngroups // SUPER)
    PADROWS = ngroups * GROUP * K * C     # partial rows (incl pad tiles)
    OUTROWS = PADROWS // NCORES
    nc = bacc.Bacc("TRN2", target_bir_lowering=False, debug=False,
                   num_devices=NCORES)
    xT_d = nc.dram_tensor("xT", [DIN, NODE_PAD], BF16, kind="ExternalInput")
    w_d = nc.dram_tensor("w33", [DIN, GF], BF16, kind="ExternalInput")
    b_d = nc.dram_tensor("b33", [1, GF], BF16, kind="ExternalInput")
    a2_d = nc.dram_tensor("a233", [DOUT, GF], BF16, kind="ExternalInput")
    wa2_d = nc.dram_tensor("wa233", [DIN, GF], BF16, kind="ExternalInput")
    c_d = nc.dram_tensor("c33", [1, GF], BF16, kind="ExternalInput")
    slots_d = nc.dram_tensor("slots", [128, ntiles * IPP], I16,
                             kind="ExternalInput")
    lhs_d = nc.dram_tensor("lhs", [128, ntiles * K], BF16,
                           kind="ExternalInput")
    out_d = nc.dram_tensor("out", [OUTROWS, GF], BF16, kind="ExternalOutput")

    with tile.TileContext(nc) as tc:
        with tc.tile_pool(name="const", bufs=1) as constp, \
             tc.tile_pool(name="xp", bufs=3) as xp, \
             tc.tile_pool(name="work", bufs=3) as work, \
             tc.tile_pool(name="big", bufs=1) as big, \
             tc.tile_pool(name="gbuf", bufs=6) as gbuf, \
             tc.tile_pool(name="ibuf", bufs=3) as ibuf, \
             tc.tile_pool(name="ev", bufs=3) as evp, \
             tc.tile_pool(name="ps", bufs=2, space="PSUM") as ps, \
             tc.tile_pool(name="pst", bufs=2, space="PSUM") as pst, \
             tc.tile_pool(name="pse", bufs=2, space="PSUM") as pse, \
             tc.tile_pool(name="pse2", bufs=2, space="PSUM") as pse2, \
             tc.tile_pool(name="dram", bufs=1, space="DRAM") as dram:

            # ---- consts, one per queue so they load in parallel ----
            w_sb = constp.tile([DIN, GF], BF16)
            nc.sync.dma_start(out=w_sb[:], in_=w_d[:, :])
            b_sb = constp.tile([1, GF], BF16)
            nc.scalar.dma_start(out=b_sb[:], in_=b_d[:, :])
            a2_sb = constp.tile([DOUT, GF], BF16)
            nc.gpsimd.dma_start(out=a2_sb[:], in_=a2_d[:, :])
            wa2_sb = constp.tile([DIN, GF], BF16)
            nc.gpsimd.dma_start(out=wa2_sb[:], in_=wa2_d[:, :])
            c_sb = constp.tile([1, GF], BF16)
            nc.sync.dma_start(out=c_sb[:], in_=c_d[:, :])
            ident = constp.tile([128, 128], BF16)
            make_identity(nc, ident[:])
            ones_sb = constp.tile([1, MMCHUNK], BF16)
            nc.gpsimd.memset(ones_sb[:], 1.0)
            # edge-phase profiles: split the load across three queues
            lhs_sb = constp.tile([128, ntiles * K], BF16)
            third = (ntiles * K) // 3
            nc.sync.dma_start(out=lhs_sb[:, 0:third],
                              in_=lhs_d[:, 0:third])
            nc.scalar.dma_start(out=lhs_sb[:, third:2 * third],
                                in_=lhs_d[:, third:2 * third])
            nc.gpsimd.dma_start(out=lhs_sb[:, 2 * third:],
                                in_=lhs_d[:, 2 * third:])

            # ---- node phase: gT[f, n] = [p*h; p] (feature-major, bf16) ----
            # hps rows 0:32 = W@x + b, row 32 = 1.0 (w col 32 = 0, b col 32
            # = 1), so gT = hps * p has row 32 = p.
            gT = big.tile([GF, NODE_PAD], BF16, tag="gT")
            for t in range(NCHUNKS):
                cs = slice(t * MMCHUNK, (t + 1) * MMCHUNK)
                xt = xp.tile([DIN, MMCHUNK], BF16)
                nc.sync.dma_start(out=xt[:], in_=xT_d[:, cs])
                hps = ps.tile([GF, MMCHUNK], F32, space="PSUM")
                nc.tensor.matmul(hps[:], lhsT=w_sb[:],
                                 rhs=xt[:], start=True, stop=False)
                nc.tensor.matmul(hps[:], lhsT=b_sb[:],
                                 rhs=ones_sb[:], start=False, stop=True)
                # leaky(h) = slope*h + (1-slope)*relu(h); the slope*h term
                # is linear in x so it folds into the s2 matmul via
                # wa2 = W^T a2 (host-precomputed const):
                # s2 = slope*(wa2^T x) + (1-slope)*(a2^T relu(h)) + slope*a2.b
                r_sb = work.tile([DOUT, MMCHUNK], BF16, tag="r")
                nc.scalar.activation(out=r_sb[:], in_=hps[0:DOUT, :],
                                     func=mybir.ActivationFunctionType.Relu)
                sps = pse.tile([GF, MMCHUNK], F32, space="PSUM", tag="s2")
                nc.tensor.matmul(sps[:], lhsT=wa2_sb[:], rhs=xt[:],
                                 start=True, stop=False)
                nc.tensor.matmul(sps[:], lhsT=a2_sb[:], rhs=r_sb[:],
                                 start=False, stop=False)
                nc.tensor.matmul(sps[:], lhsT=c_sb[:], rhs=ones_sb[:],
                                 start=False, stop=True)
                p_sb = work.tile([GF, MMCHUNK], BF16, tag="p")
                nc.scalar.activation(out=p_sb[:], in_=sps[:],
                                     func=mybir.ActivationFunctionType.Exp)
                nc.vector.tensor_tensor(
                    out=gT[:, cs], in0=hps[:], in1=p_sb[:],
                    op=mybir.AluOpType.mult)
            # zero pad columns (incl ZROW, the pad-slot target)
            nc.vector.memset(gT[:, NPC:NODE_PAD], 0.0)

            # ---- transpose gT -> node-major bf16, write strided table ----
            ntile128 = NODE_PAD // 128
            g_sb = big.tile([128, ntile128 * GF], BF16)
            # pack 10 transposes into one PSUM tile, evac with one wide copy
            # pack 10 transposes into one PSUM tile (stride 34 to keep PSUM
            # writes 4B-aligned), evac with one wide strided copy
            TB = 10
            GFP = GF + 1
            for tb in range(ntile128 // TB):
                tp = pst.tile([128, TB * GFP], BF16, space="PSUM")
                for u in range(TB):
                    t = tb * TB + u
                    nc.tensor.transpose(
                        out=tp[:, u * GFP:u * GFP + GF],
                        in_=gT[:, t * 128:(t + 1) * 128],
                        identity=ident[:GF, :GF])
                base = tb * TB * GF
                src = tp[:].rearrange("p (t f) -> p t f", f=GFP)[:, :, 0:GF]
                dst = g_sb[:, base:base + TB * GF].rearrange(
                    "p (t f) -> p t f", f=GF)
                if tb % 2 == 0:
                    nc.vector.tensor_copy(out=dst, in_=src)
                else:
                    nc.scalar.activation(
                        out=dst, in_=src,
                        func=mybir.ActivationFunctionType.Copy)
            tbl = dram.tile([NODE_PAD, GSTRIDE], BF16)
            half = ntile128 // 2
            nc.sync.dma_start(
                out=tbl[0:half * 128, 0:GF]
                    .rearrange("(t p) f -> p t f", p=128),
                in_=g_sb[:, 0:half * GF]
                    .rearrange("p (t f) -> p t f", f=GF))
            nc.scalar.dma_start(
                out=tbl[half * 128:, 0:GF]
                    .rearrange("(t p) f -> p t f", p=128),
                in_=g_sb[:, half * GF:]
                    .rearrange("p (t f) -> p t f", f=GF))

            # ---- edge phase: gather + matmul-reduce ----
            bal = _Balance(nc)
            GATHER_COST = {POOL: 413}   # SWDGE ucode is Pool-only on HW
            EVAC_COST = {ACT: 597, DVE: 641}
            DMA_COST = {ACT: 770, SP: 770}   # keep Pool free for gathers
            part_d = dram.tile([PADROWS, GF], BF16)
            for s in range(nsuper):
                g_lo = s * SUPER
                g_hi = min((s + 1) * SUPER, ngroups)
                ngr = g_hi - g_lo
                idx = ibuf.tile([128, ngr * GROUP * IPP], I16, tag="idx")
                t_lo = g_lo * GROUP
                n_idx_tiles = min(ngr * GROUP, ntiles - t_lo)
                eng, _ = bal.pick(DMA_COST)
                eng.dma_start(
                    out=idx[:, 0:n_idx_tiles * IPP],
                    in_=slots_d[:, t_lo * IPP:(t_lo + n_idx_tiles) * IPP])
                ev = evp.tile([96, ngr * C * GF], BF16, tag="ev")
                for gi in range(ngr):
                    g = g_lo + gi
                    out_ps = pse2.tile([128, 512], F32, space="PSUM",
                                       tag="eps")
                    for j in range(GROUP):
                        t = g * GROUP + j
                        if t < ntiles:
                            B = gbuf.tile([128, C * GF], BF16, tag="B")
                            geng, _ = bal.pick(GATHER_COST)
                            dma_gather_raw(
                                nc, B[:].rearrange("p (c f) -> p c f", f=GF),
                                tbl[:, 0:GF],
                                idx[:, (gi * GROUP + j) * IPP:
                                       (gi * GROUP + j + 1) * IPP],
                                NI, GF, GSTRIDE, eng=geng)
                            nc.tensor.matmul(
                                out=out_ps[j * K:(j + 1) * K, 0:C * GF],
                                lhsT=lhs_sb[:, t * K:(t + 1) * K],
                                rhs=B[:], start=True, stop=True)
                        else:
                            nc.vector.memset(
                                out_ps[j * K:(j + 1) * K, 0:C * GF], 0.0)
                    eeng, eid = bal.pick(EVAC_COST)
                    dst = ev[:, gi * C * GF:(gi + 1) * C * GF]
                    if eid == ACT:
                        nc.scalar.activation(
                            out=dst, in_=out_ps[0:96, 0:C * GF],
                            func=mybir.ActivationFunctionType.Copy)
                    else:
                        eeng.tensor_copy(out=dst, in_=out_ps[0:96, 0:C * GF])
                eng, _ = bal.pick(DMA_COST)
                eng.dma_start(
                    out=part_d[g_lo * GROUP * K * C:
                               (g_lo + ngr) * GROUP * K * C, :]
                        .rearrange("(g j i c) f -> (j i) g c f",
                                   g=ngr, j=GROUP, c=C),
                    in_=ev[:].rearrange("p (g c f) -> p g c f",
                                        g=ngr, f=GF))

            # ---- ReduceScatter partials -> my output shard ----
            rs_out = dram.tile([OUTROWS, GF], BF16)
            nc.gpsimd.collective_compute(
                "ReduceScatter", mybir.AluOpType.add,
                ins=[part_d[:].opt()], outs=[rs_out[:].opt()],
                replica_groups=[list(range(NCORES))])
            # DRAM->DRAM copies price poorly; bounce through SBUF
            pdim = max(d for d in range(1, 129) if OUTROWS % d == 0)
            ob = evp.tile([pdim, (OUTROWS // pdim) * GF], BF16, tag="ob")
            nc.sync.dma_start(
                out=ob[:],
                in_=rs_out[:, :].rearrange("(p c) f -> p (c f)", p=pdim))
            nc.scalar.dma_start(
                out=out_d[:, :].rearrange("(p c) f -> p (c f)", p=pdim),
                in_=ob[:])

    nc.compile()
    return nc


class _Runner:
    """shard_map-jitted executor (mirrors bass2jax.run_bass_via_pjrt)."""

    def __init__(self, nc, n_cores):
        install_neuronx_cc_hook()
        self.n_cores = n_cores
        partition_name = (nc.partition_id_tensor.name
                          if nc.partition_id_tensor else None)
        in_names, out_names, out_avals, zero_outs = [], [], [], []
        for alloc in nc.m.functions[0].allocations:
            if not isinstance(alloc, mybir.MemoryLocationSet):
                continue
            name = alloc.memorylocations[0].name
            if alloc.kind == "ExternalInput":
                if name != partition_name:
                    in_names.append(name)
            elif alloc.kind == "ExternalOutput":
                out_names.append(name)
                shape = tuple(alloc.tensor_shape)
                dtype = mybir.dt.np(alloc.dtype)
                out_avals.append(jax.core.ShapedArray(shape, dtype))
                zero_outs.append(np.zeros(shape, dtype))
        self.in_names = in_names
        self.out_names = out_names
        self.out_avals = out_avals
        self.zero_outs = zero_outs
        n_params = len(in_names)
        self.n_params = n_params
        all_in = in_names + out_names
        if partition_name is not None:
            all_in.append(partition_name)
        donate = tuple(range(n_params, n_params + len(out_avals)))

        def _body(*args):
            operands = list(args)
            if partition_name is not None:
                operands.append(bass2jax.partition_id_tensor())
            outs = _bass_exec_p.bind(
                *operands, out_avals=tuple(out_avals),
                in_names=tuple(all_in), out_names=tuple(out_names),
                lowering_input_output_aliases=(),
                sim_require_finite=True, sim_require_nnan=True, nc=nc)
            return tuple(outs)

        devices = jax.devices()[:n_cores]
        mesh = Mesh(np.asarray(devices), ("core",))
        self._fn = jax.jit(
            shard_map(_body, mesh=mesh,
                      in_specs=(PartitionSpec("core"),) * (n_params +
                                                           len(out_avals)),
                      out_specs=(PartitionSpec("core"),) * len(out_names),
                      check_rep=False),
            donate_argnums=donate, keep_unused=True)

    def run(self, in_maps):
        per_core = [[np.asarray(m[n]) for n in self.in_names]
                    for m in in_maps]
        concat_in = [
            np.concatenate([per_core[c][i] for c in range(self.n_cores)],
                           axis=0)
            for i in range(self.n_params)
        ]
        concat_zeros = [
            np.zeros((self.n_cores * z.shape[0], *z.shape[1:]), z.dtype)
            for z in self.zero_outs
        ]
        out_arrs = self._fn(*concat_in, *concat_zeros)
        jax.block_until_ready(out_arrs)
        return [
            {name: np.asarray(out_arrs[i]).reshape(
                self.n_cores, *self.out_avals[i].shape)[c]
             for i, name in enumerate(self.out_names)}
            for c in range(self.n_cores)
        ]


_CACHE = {}


def _consts(W1_w, W1_b, a2_w):
    import ml_dtypes
    W = np.asarray(W1_w).astype(np.float32)
    b = np.asarray(W1_b).astype(np.float32)
    a2 = np.asarray(a2_w).astype(np.float32)
    w33 = np.zeros((DIN, GF), np.float32)
    w33[:, 0:DOUT] = W.T
    b33 = np.zeros((1, GF), np.float32)
    b33[0, 0:DOUT] = b
    b33[0, DOUT] = 1.0
    a233 = np.repeat(((1.0 - SLOPE) * a2).reshape(DOUT, 1), GF, axis=1)
    wa2 = SLOPE * (W.T @ a2)                       # [DIN]
    wa233 = np.repeat(wa2.reshape(DIN, 1), GF, axis=1)
    c33 = np.full((1, GF), SLOPE * float(b @ a2), np.float32)
    return {
        "w33": w33.astype(ml_dtypes.bfloat16),
        "b33": b33.astype(ml_dtypes.bfloat16),
        "a233": a233.astype(ml_dtypes.bfloat16),
        "wa233": wa233.astype(ml_dtypes.bfloat16),
        "c33": c33.astype(ml_dtypes.bfloat16),
    }


def _get_runner(ntiles):
    if ntiles not in _CACHE:
        nc = _build_program(ntiles)
        _CACHE[ntiles] = (nc, _Runner(nc, NCORES))
    return _CACHE[ntiles]


def kernel(x, edge_index, W1_w, W1_b, a1_w=None, a2_w=None):
    ntiles, cell_map, per_core = _host_shard(x, edge_index)
    nc, runner = _get_runner(ntiles)
    consts = _consts(W1_w, W1_b, a2_w)
    in_maps = [{**per_core[c], **consts} for c in range(NCORES)]
    results = runner.run(in_maps)
    full = np.concatenate([results[c]["out"] for c in range(NCORES)], axis=0)
    ncells = ntiles * K * C
    cm = np.concatenate([cell_map,
                         np.full(full.shape[0] - ncells, -1, np.int64)])
    valid = cm >= 0
    out = np.empty((N, DOUT), np.float32)
    rows = full[valid].astype(np.float32)
    out[cm[valid]] = rows[:, 0:DOUT] / rows[:, DOUT:DOUT + 1]
    return out
